# revision 17
# baseline (speedup 1.0000x reference)
"""Trainium2 Bass kernel for BboxRegression (topk_masking).

reference:
    fcn  = einsum("bnd,df->bnf", x_out, W) + b          # [B, N, 4]
    idx  = argmax(ref_scores, axis=1)                   # [B]
    bbox = fcn[arange(B), idx, :]                       # [B, 4]
    slice_inds = stack([arange(B), idx], 1)             # [B, 2] int32

B, N, D = 4096, 128, 512.  Pure data parallel over 8 cores: each core gets
512 batches (= 65536 rows of 512 fp32 = 128 MiB).  Memory-bound problem.

Per-core device pipeline (all fp32 data, fp32r on the PE):
  - stream x in [128, 2048] SBUF tiles (4 row-tiles of 128 rows x 512 d)
  - PE transposes each [128,128] chunk into PSUM (d on partitions)
  - ACT copies PSUM -> SBUF
  - PE matmul: W chunk [128, 4] stationary, xT [128, 512] moving, PSUM accum
  - DVE adds bias + copies PSUM -> SBUF
  - DMA out fcn in transposed [4, 65536] layout (contiguous 2 KB runs);
    the host un-transposes while unsharding.
  - argmax: nc.vector.max / max_index on [128, 128] score tiles; winning x
    rows re-fetched with indirect DMA and pushed through the same
    transpose+project pipeline -> bbox in [4, 512] layout.
"""

import os
import sys

import numpy as np

sys.path.insert(0, "/opt/trn_rl_repo")

os.environ.setdefault("MYCRO_LOCAL_CACHE", "1")

import concourse.bass as bass  # noqa: E402
import concourse.mybir as mybir  # noqa: E402
import concourse.tile as tile  # noqa: E402
from concourse.bass_utils import run_bass_kernel_spmd  # noqa: E402

B, N, D, F = 4096, 128, 512, 4
N_CORES = 8
B_LOC = B // N_CORES              # 512 batches per core
ROWS = B_LOC * N                  # 65536 rows per core
GROUP = 4                         # row-tiles (of 128 rows) per group
GROUP_ROWS = GROUP * 128          # 512 rows per group
N_GROUPS = ROWS // GROUP_ROWS     # 128 groups
N_SC_TILES = B_LOC // 128         # 4 score tiles of [128 b, 128 n]

f32 = mybir.dt.float32
f32r = mybir.dt.float32r
i32 = mybir.dt.int32
u32 = mybir.dt.uint32

LAST_RESULTS = None  # BassKernelResults of the most recent run (for test.py)


def _split_drain_waits(nc):
    """Walrus in this container rejects ANY instruction carrying more than one
    sync wait ("Too many sync wait commands", CoreV3GenImpl setupSyncWait);
    Tile's add_semaphores freely attaches several.  Hoist the extra waits onto
    standalone EventSemaphore instructions (1 wait each) inserted just before
    the instruction on the same engine — same AND semantics in program order."""
    n = 0
    for fn in nc.m.functions:
        for bb in fn.blocks:
            new_insts = []
            for inst in bb.instructions:
                si = inst.sync_info
                if si is not None and len(si.on_wait) > 1:
                    waits = list(si.on_wait)
                    for w in waits[:-1]:
                        ev = mybir.InstEventSemaphore(
                            name=f"{inst.name}_splitw{n}", ins=[], outs=[]
                        )
                        n += 1
                        ev.engine = inst.engine
                        ev.sync_info = mybir.SyncInfo(on_wait=[w], on_update=[])
                        new_insts.append(ev)
                    inst.sync_info = mybir.SyncInfo(
                        on_wait=[waits[-1]], on_update=list(si.on_update)
                    )
                new_insts.append(inst)
            bb.instructions[:] = new_insts


def _build_bass():
    nc = bass.Bass()

    x = nc.declare_dram_parameter("x", [ROWS, D], f32r, isOutput=False)
    scores = nc.declare_dram_parameter("scores", [B_LOC, N], f32, isOutput=False)
    w_in = nc.declare_dram_parameter("w", [D, F], f32, isOutput=False)
    b_in = nc.declare_dram_parameter("bias", [F, 1], f32, isOutput=False)
    ident_in = nc.declare_dram_parameter("ident", [128, 128], f32r, isOutput=False)

    fcn_t = nc.declare_dram_parameter("fcn_t", [F, ROWS], f32, isOutput=True)
    bbox_t = nc.declare_dram_parameter("bbox_t", [F, B_LOC], f32, isOutput=True)
    idx_out = nc.declare_dram_parameter("idx", [B_LOC, 1], i32, isOutput=True)

    DC = D // 128  # 4 contraction chunks

    with tile.TileContext(nc) as tc:
        with (
            tc.tile_pool(name="consts", bufs=1) as cpool,
            tc.tile_pool(name="xin", bufs=7) as xin_pool,
            tc.tile_pool(name="xt_ps", bufs=6, space="PSUM") as xtps_pool,
            tc.tile_pool(name="y_ps", bufs=2, space="PSUM") as yps_pool,
            tc.tile_pool(name="xt_sb", bufs=6) as xtsb_pool,
            tc.tile_pool(name="y_sb", bufs=4) as ysb_pool,
            tc.tile_pool(name="sc", bufs=4) as sc_pool,
            tc.tile_pool(name="gx", bufs=4) as gx_pool,
            tc.tile_pool(name="small", bufs=8) as sm_pool,
        ):
            # constants
            ident = cpool.tile_from(ident_in[:, :])
            w_stage = cpool.tile([128, DC, F], f32)
            nc.sync.dma_start(
                out=w_stage[:, :, :],
                in_=w_in.rearrange("(c p) f -> p c f", c=DC, p=128),
            )
            # fp32r matmul operands must be produced rounded-to-fp32r
            w_sb = cpool.tile([128, DC, F], f32r)
            nc.vector.tensor_copy(out=w_sb[:, :, :], in_=w_stage[:, :, :])
            bias_sb = cpool.tile_from(b_in[:, :])

            def emit_group(in_slice, y_dram_cols):
                """in_slice(g, c) -> [128 rows, 128 d] fp32 SBUF AP for
                row-tile g, d-chunk c.  Projects 512 rows -> writes
                y_dram_cols (an AP of fcn_t/bbox_t columns)."""
                y_ps = yps_pool.tile([F, GROUP_ROWS], f32)
                for c in range(DC):
                    xt_ps = xtps_pool.tile([128, GROUP_ROWS], f32r, tag="xt")
                    for g in range(GROUP):
                        nc.tensor.transpose(
                            out=xt_ps[:, g * 128:(g + 1) * 128],
                            in_=in_slice(g, c),
                            identity=ident[:, :],
                        )
                    # PSUM->SBUF copy doubles as the round-to-fp32r; split the
                    # four chunk copies between ACT and DVE (each is 1x-rate)
                    xt_sb = xtsb_pool.tile([128, GROUP_ROWS], f32r, tag="xt_sb")
                    if c % 2 == 0:
                        nc.scalar.copy(out=xt_sb[:, :], in_=xt_ps[:, :])
                    else:
                        nc.vector.tensor_copy(out=xt_sb[:, :], in_=xt_ps[:, :])
                    nc.tensor.matmul(
                        out=y_ps[:, :],
                        lhsT=w_sb[:, c, :],
                        rhs=xt_sb[:, :],
                        start=(c == 0),
                        stop=(c == DC - 1),
                    )
                y_sb = ysb_pool.tile([F, GROUP_ROWS], f32)
                nc.vector.tensor_scalar(
                    out=y_sb[:, :],
                    in0=y_ps[:, :],
                    scalar1=bias_sb[:, :1],
                    scalar2=None,
                    op0=mybir.AluOpType.add,
                )
                # stores ride the ACT-issued HWDGE ring so they don't queue
                # behind the next x load on the SP ring
                nc.scalar.dma_start(out=y_dram_cols, in_=y_sb[:, :])

            def emit_tp(TP):
                xbuf = xin_pool.tile([128, 2 * GROUP, D], f32r)
                # alternate between the two HWDGE rings (SP / ACT) so the 16
                # SDMA engines have descriptors from two queues in flight
                ld_eng = nc.sync if TP % 2 == 0 else nc.scalar
                ld_eng.dma_start(
                    out=xbuf[:, :, :],
                    in_=x[
                        TP * 2 * GROUP_ROWS:(TP + 1) * 2 * GROUP_ROWS, :
                    ].rearrange("(p r) d -> p r d", p=128, r=2 * GROUP),
                )
                for h in range(2):
                    T = 2 * TP + h
                    emit_group(
                        lambda g, c, xbuf=xbuf, h=h: xbuf[
                            :, h * GROUP + g, c * 128:(c + 1) * 128
                        ],
                        fcn_t[:, T * GROUP_ROWS:(T + 1) * GROUP_ROWS],
                    )

            HEAD_TPS = 4
            for TP in range(HEAD_TPS):
                emit_tp(TP)

            # ---- scores: per-row argmax + gather of winning x rows ----
            gx_tiles = []
            for t in range(N_SC_TILES):
                sc = sc_pool.tile([128, N], f32)
                nc.sync.dma_start(
                    out=sc[:, :], in_=scores[t * 128:(t + 1) * 128, :]
                )
                m8 = sm_pool.tile([128, 8], f32, tag="m8")
                i8 = sm_pool.tile([128, 8], u32, tag="i8")
                nc.vector.max(out=m8[:, :], in_=sc[:, :])
                nc.vector.max_index(out=i8[:, :], in_max=m8[:, :], in_values=sc[:, :])
                idx_i = sm_pool.tile([128, 1], i32, tag="idxi")
                nc.vector.tensor_copy(out=idx_i[:, :], in_=i8[:, :1])
                nc.scalar.dma_start(
                    out=idx_out[t * 128:(t + 1) * 128, :], in_=idx_i[:, :]
                )
                # global row index of the winning row: (t*128 + p) * 128 + idx
                base = sm_pool.tile([128, 1], i32, tag="base")
                nc.gpsimd.iota(
                    out=base[:, :],
                    pattern=[[0, 1]],
                    base=t * 128 * N,
                    channel_multiplier=N,
                )
                gidx = sm_pool.tile([128, 1], i32, tag="gidx")
                nc.vector.tensor_tensor(
                    out=gidx[:, :], in0=base[:, :], in1=idx_i[:, :],
                    op=mybir.AluOpType.add,
                )
                gx = gx_pool.tile([128, D], f32r)
                nc.gpsimd.indirect_dma_start(
                    out=gx[:, :],
                    out_offset=None,
                    in_=x[:, :],
                    in_offset=bass.IndirectOffsetOnAxis(ap=gidx[:, :1], axis=0),
                )
                gx_tiles.append(gx)

            # ---- bbox: project the gathered winner rows ----
            emit_group(
                lambda g, c: gx_tiles[g][:, c * 128:(c + 1) * 128],
                bbox_t[:, :],
            )

            # ---- main projection over all remaining rows ----
            # 2 MiB loads; each SBUF partition holds 8 CONSECUTIVE rows so the
            # DMA does one contiguous 16 KB DRAM run per partition.  Row p*8+r
            # lands at xbuf[p, r, :]; the host un-permutes fcn_t's column
            # order ((h,g,p) -> p*8+h*4+g) while unsharding.
            for TP in range(HEAD_TPS, N_GROUPS // 2):
                emit_tp(TP)

    _split_drain_waits(nc)
    return nc


_NC_CACHE = None


def _get_nc():
    global _NC_CACHE
    if _NC_CACHE is None:
        _NC_CACHE = _build_bass()
    return _NC_CACHE


def _install_profile_shim():
    """Register the NTFF profile hook that the agent image's antenv lacks.

    Mirrors trn_boot._ntff_profile_via_ctypes: drives NRT profiling through
    libaxon_pjrt.so's C ABI so run_bass_kernel_spmd(trace=True) can capture
    per-instruction timelines. No-op if already present or the .so is old.
    """
    import contextlib
    import ctypes
    import types

    try:
        from antenv.axon_hooks import get_axon_ntff_profile_hook  # noqa: F401
        return
    except ImportError:
        pass

    so_path = "/opt/axon/libaxon_pjrt.so"
    if not os.path.exists(so_path):
        return
    lib = ctypes.CDLL(so_path)
    if not hasattr(lib, "axon_start_nrt_profile"):
        return
    lib.axon_start_nrt_profile.argtypes = [
        ctypes.POINTER(ctypes.c_int64),
        ctypes.c_size_t,
    ]
    lib.axon_start_nrt_profile.restype = ctypes.c_int64
    lib.axon_stop_nrt_profile.argtypes = [ctypes.c_char_p]
    lib.axon_stop_nrt_profile.restype = ctypes.c_int64

    @contextlib.contextmanager
    def _hook(output_dir, device_ids):
        import jax

        jax.devices()
        if device_ids:
            ids = (ctypes.c_int64 * len(device_ids))(*device_ids)
            rc = lib.axon_start_nrt_profile(ids, len(device_ids))
        else:
            rc = lib.axon_start_nrt_profile(None, 0)
        if rc != 0:
            raise RuntimeError(f"axon_start_nrt_profile rc={rc}")
        try:
            yield
        finally:
            n = lib.axon_stop_nrt_profile(str(output_dir).encode())
            print(f"profile: {n} file(s) written to {output_dir}", file=sys.stderr)

    import antenv

    mod = types.ModuleType("antenv.axon_hooks")
    mod.get_axon_ntff_profile_hook = lambda: _hook
    sys.modules["antenv.axon_hooks"] = mod
    antenv.axon_hooks = mod


def kernel(x_out, ref_scores, W, b):
    global LAST_RESULTS
    x_out = np.ascontiguousarray(x_out, dtype=np.float32)
    ref_scores = np.ascontiguousarray(ref_scores, dtype=np.float32)
    W = np.ascontiguousarray(W, dtype=np.float32)
    b = np.ascontiguousarray(b, dtype=np.float32)

    ident = np.eye(128, dtype=np.float32)
    in_maps = []
    for c in range(N_CORES):
        bs = slice(c * B_LOC, (c + 1) * B_LOC)
        in_maps.append({
            "x": x_out[bs].reshape(ROWS, D),
            "scores": ref_scores[bs],
            "w": W,
            "bias": b.reshape(F, 1),
            "ident": ident,
        })

    trace = bool(int(os.environ.get("BASS_KERNEL_TRACE", "0")))
    if trace:
        _install_profile_shim()
    nc = _get_nc()
    res = run_bass_kernel_spmd(
        nc,
        in_maps,
        list(range(N_CORES)),
        trace=trace,
        tmpdir=os.environ.get("BASS_KERNEL_TRACE_DIR") or None,
    )
    LAST_RESULTS = res

    def _unpermute_fcn(fcn_t):
        # device column order is (TP, h, g, p) <-> row TP*1024 + p*8 + h*4 + g
        a = fcn_t.reshape(F, N_GROUPS // 2, 2, GROUP, 128)
        return np.ascontiguousarray(
            a.transpose(1, 4, 2, 3, 0).reshape(ROWS, F)
        )

    fcn = np.concatenate(
        [_unpermute_fcn(res.results[c]["fcn_t"]) for c in range(N_CORES)], axis=0
    ).reshape(B, N, F)
    bbox = np.concatenate(
        [res.results[c]["bbox_t"].T for c in range(N_CORES)], axis=0
    )
    idx = np.concatenate(
        [res.results[c]["idx"][:, 0] for c in range(N_CORES)], axis=0
    ).astype(np.int32)
    slice_inds = np.stack([np.arange(B, dtype=np.int32), idx], axis=1)
    return bbox, fcn, slice_inds


# revision 18
# speedup vs baseline: 1.1717x; 1.1717x over previous
"""Trainium2 Bass kernel for BboxRegression (topk_masking).

reference:
    fcn  = einsum("bnd,df->bnf", x_out, W) + b          # [B, N, 4]
    idx  = argmax(ref_scores, axis=1)                   # [B]
    bbox = fcn[arange(B), idx, :]                       # [B, 4]
    slice_inds = stack([arange(B), idx], 1)             # [B, 2] int32

B, N, D = 4096, 128, 512.  Pure data parallel over 8 cores: each core gets
512 batches (= 65536 rows of 512 fp32 = 128 MiB).  Memory-bound problem.

Per-core device pipeline (all fp32 data, fp32r on the PE):
  - stream x in [128, 2048] SBUF tiles (4 row-tiles of 128 rows x 512 d)
  - PE transposes each [128,128] chunk into PSUM (d on partitions)
  - ACT copies PSUM -> SBUF
  - PE matmul: W chunk [128, 4] stationary, xT [128, 512] moving, PSUM accum
  - DVE adds bias + copies PSUM -> SBUF
  - DMA out fcn in transposed [4, 65536] layout (contiguous 2 KB runs);
    the host un-transposes while unsharding.
  - argmax: nc.vector.max / max_index on [128, 128] score tiles; winning x
    rows re-fetched with indirect DMA and pushed through the same
    transpose+project pipeline -> bbox in [4, 512] layout.
"""

import os
import sys

import numpy as np

sys.path.insert(0, "/opt/trn_rl_repo")

os.environ.setdefault("MYCRO_LOCAL_CACHE", "1")

import concourse.bass as bass  # noqa: E402
import concourse.mybir as mybir  # noqa: E402
import concourse.tile as tile  # noqa: E402
from concourse.bass_utils import run_bass_kernel_spmd  # noqa: E402

B, N, D, F = 4096, 128, 512, 4
N_CORES = 8
B_LOC = B // N_CORES              # 512 batches per core
ROWS = B_LOC * N                  # 65536 rows per core
GROUP = 4                         # row-tiles (of 128 rows) per group
GROUP_ROWS = GROUP * 128          # 512 rows per group
N_GROUPS = ROWS // GROUP_ROWS     # 128 groups
N_SC_TILES = B_LOC // 128         # 4 score tiles of [128 b, 128 n]

f32 = mybir.dt.float32
f32r = mybir.dt.float32r
i32 = mybir.dt.int32
u32 = mybir.dt.uint32

LAST_RESULTS = None  # BassKernelResults of the most recent run (for test.py)


def _split_drain_waits(nc):
    """Walrus in this container rejects ANY instruction carrying more than one
    sync wait ("Too many sync wait commands", CoreV3GenImpl setupSyncWait);
    Tile's add_semaphores freely attaches several.  Hoist the extra waits onto
    standalone EventSemaphore instructions (1 wait each) inserted just before
    the instruction on the same engine — same AND semantics in program order."""
    n = 0
    for fn in nc.m.functions:
        for bb in fn.blocks:
            new_insts = []
            for inst in bb.instructions:
                si = inst.sync_info
                if si is not None and len(si.on_wait) > 1:
                    waits = list(si.on_wait)
                    for w in waits[:-1]:
                        ev = mybir.InstEventSemaphore(
                            name=f"{inst.name}_splitw{n}", ins=[], outs=[]
                        )
                        n += 1
                        ev.engine = inst.engine
                        ev.sync_info = mybir.SyncInfo(on_wait=[w], on_update=[])
                        new_insts.append(ev)
                    inst.sync_info = mybir.SyncInfo(
                        on_wait=[waits[-1]], on_update=list(si.on_update)
                    )
                new_insts.append(inst)
            bb.instructions[:] = new_insts


def _build_bass():
    nc = bass.Bass()

    x = nc.declare_dram_parameter("x", [ROWS, D], f32r, isOutput=False)
    scores = nc.declare_dram_parameter("scores", [B_LOC, N], f32, isOutput=False)
    w_in = nc.declare_dram_parameter("w", [D, F], f32, isOutput=False)
    b_in = nc.declare_dram_parameter("bias", [F, 1], f32, isOutput=False)
    ident_in = nc.declare_dram_parameter("ident", [128, 128], f32r, isOutput=False)

    fcn_t = nc.declare_dram_parameter("fcn_t", [F, ROWS], f32, isOutput=True)
    bbox_t = nc.declare_dram_parameter("bbox_t", [F, B_LOC], f32, isOutput=True)
    idx_out = nc.declare_dram_parameter("idx", [B_LOC, 1], i32, isOutput=True)

    DC = D // 128  # 4 contraction chunks

    with tile.TileContext(nc) as tc:
        with (
            tc.tile_pool(name="consts", bufs=1) as cpool,
            tc.tile_pool(name="xin", bufs=5) as xin_pool,
            tc.tile_pool(name="xt_ps", bufs=6, space="PSUM") as xtps_pool,
            tc.tile_pool(name="y_ps", bufs=2, space="PSUM") as yps_pool,
            tc.tile_pool(name="xt_sb", bufs=6) as xtsb_pool,
            tc.tile_pool(name="y_sb", bufs=4) as ysb_pool,
            tc.tile_pool(name="sc", bufs=4) as sc_pool,
            tc.tile_pool(name="gx", bufs=4) as gx_pool,
            tc.tile_pool(name="small", bufs=8) as sm_pool,
        ):
            # constants
            ident = cpool.tile_from(ident_in[:, :])
            w_stage = cpool.tile([128, DC, F], f32)
            nc.sync.dma_start(
                out=w_stage[:, :, :],
                in_=w_in.rearrange("(c p) f -> p c f", c=DC, p=128),
            )
            # fp32r matmul operands must be produced rounded-to-fp32r
            w_sb = cpool.tile([128, DC, F], f32r)
            nc.vector.tensor_copy(out=w_sb[:, :, :], in_=w_stage[:, :, :])
            bias_sb = cpool.tile_from(b_in[:, :])

            def emit_group(in_slice, y_dram_cols):
                """in_slice(g, c) -> [128 rows, 128 d] fp32 SBUF AP for
                row-tile g, d-chunk c.  Projects 512 rows -> writes
                y_dram_cols (an AP of fcn_t/bbox_t columns)."""
                y_ps = yps_pool.tile([F, GROUP_ROWS], f32)
                for c in range(DC):
                    xt_ps = xtps_pool.tile([128, GROUP_ROWS], f32r, tag="xt")
                    for g in range(GROUP):
                        nc.tensor.transpose(
                            out=xt_ps[:, g * 128:(g + 1) * 128],
                            in_=in_slice(g, c),
                            identity=ident[:, :],
                        )
                    # PSUM->SBUF copy doubles as the round-to-fp32r; split the
                    # four chunk copies between ACT and DVE (each is 1x-rate)
                    xt_sb = xtsb_pool.tile([128, GROUP_ROWS], f32r, tag="xt_sb")
                    if c % 2 == 0:
                        nc.scalar.copy(out=xt_sb[:, :], in_=xt_ps[:, :])
                    else:
                        nc.vector.tensor_copy(out=xt_sb[:, :], in_=xt_ps[:, :])
                    nc.tensor.matmul(
                        out=y_ps[:, :],
                        lhsT=w_sb[:, c, :],
                        rhs=xt_sb[:, :],
                        start=(c == 0),
                        stop=(c == DC - 1),
                    )
                y_sb = ysb_pool.tile([F, GROUP_ROWS], f32)
                nc.vector.tensor_scalar(
                    out=y_sb[:, :],
                    in0=y_ps[:, :],
                    scalar1=bias_sb[:, :1],
                    scalar2=None,
                    op0=mybir.AluOpType.add,
                )
                # stores ride the ACT-issued HWDGE ring so they don't queue
                # behind the next x load on the SP ring
                nc.scalar.dma_start(out=y_dram_cols, in_=y_sb[:, :])

            def emit_tp(TP):
                xbuf = xin_pool.tile([128, 2 * GROUP, D], f32r)
                # alternate between the two HWDGE rings (SP / ACT) so the 16
                # SDMA engines have descriptors from two queues in flight
                ld_eng = nc.sync if TP % 2 == 0 else nc.scalar
                ld_eng.dma_start(
                    out=xbuf[:, :, :],
                    in_=x[
                        TP * 2 * GROUP_ROWS:(TP + 1) * 2 * GROUP_ROWS, :
                    ].rearrange("(p r) d -> p r d", p=128, r=2 * GROUP),
                )
                for h in range(2):
                    T = 2 * TP + h
                    emit_group(
                        lambda g, c, xbuf=xbuf, h=h: xbuf[
                            :, h * GROUP + g, c * 128:(c + 1) * 128
                        ],
                        fcn_t[:, T * GROUP_ROWS:(T + 1) * GROUP_ROWS],
                    )

            # ---- scores: per-row argmax + gather of winning x rows ----
            gx_tiles = []
            for t in range(N_SC_TILES):
                sc = sc_pool.tile([128, N], f32)
                nc.sync.dma_start(
                    out=sc[:, :], in_=scores[t * 128:(t + 1) * 128, :]
                )
                m8 = sm_pool.tile([128, 8], f32, tag="m8")
                i8 = sm_pool.tile([128, 8], u32, tag="i8")
                nc.vector.max(out=m8[:, :], in_=sc[:, :])
                nc.vector.max_index(out=i8[:, :], in_max=m8[:, :], in_values=sc[:, :])
                idx_i = sm_pool.tile([128, 1], i32, tag="idxi")
                nc.vector.tensor_copy(out=idx_i[:, :], in_=i8[:, :1])
                nc.scalar.dma_start(
                    out=idx_out[t * 128:(t + 1) * 128, :], in_=idx_i[:, :]
                )
                # global row index of the winning row: (t*128 + p) * 128 + idx
                base = sm_pool.tile([128, 1], i32, tag="base")
                nc.gpsimd.iota(
                    out=base[:, :],
                    pattern=[[0, 1]],
                    base=t * 128 * N,
                    channel_multiplier=N,
                )
                gidx = sm_pool.tile([128, 1], i32, tag="gidx")
                nc.vector.tensor_tensor(
                    out=gidx[:, :], in0=base[:, :], in1=idx_i[:, :],
                    op=mybir.AluOpType.add,
                )
                gx = gx_pool.tile([128, D], f32r)
                nc.gpsimd.indirect_dma_start(
                    out=gx[:, :],
                    out_offset=None,
                    in_=x[:, :],
                    in_offset=bass.IndirectOffsetOnAxis(ap=gidx[:, :1], axis=0),
                )
                gx_tiles.append(gx)

            # ---- bbox: project the gathered winner rows ----
            emit_group(
                lambda g, c: gx_tiles[g][:, c * 128:(c + 1) * 128],
                bbox_t[:, :],
            )

            # ---- main projection over all rows ----
            # 2 MiB loads; each SBUF partition holds 8 CONSECUTIVE rows so the
            # DMA does one contiguous 16 KB DRAM run per partition.  Row p*8+r
            # lands at xbuf[p, r, :]; the host un-permutes fcn_t's column
            # order ((h,g,p) -> p*8+h*4+g) while unsharding.
            for TP in range(N_GROUPS // 2):
                emit_tp(TP)

    _split_drain_waits(nc)
    return nc


_NC_CACHE = None


def _get_nc():
    global _NC_CACHE
    if _NC_CACHE is None:
        _NC_CACHE = _build_bass()
    return _NC_CACHE


def _install_profile_shim():
    """Register the NTFF profile hook that the agent image's antenv lacks.

    Mirrors trn_boot._ntff_profile_via_ctypes: drives NRT profiling through
    libaxon_pjrt.so's C ABI so run_bass_kernel_spmd(trace=True) can capture
    per-instruction timelines. No-op if already present or the .so is old.
    """
    import contextlib
    import ctypes
    import types

    try:
        from antenv.axon_hooks import get_axon_ntff_profile_hook  # noqa: F401
        return
    except ImportError:
        pass

    so_path = "/opt/axon/libaxon_pjrt.so"
    if not os.path.exists(so_path):
        return
    lib = ctypes.CDLL(so_path)
    if not hasattr(lib, "axon_start_nrt_profile"):
        return
    lib.axon_start_nrt_profile.argtypes = [
        ctypes.POINTER(ctypes.c_int64),
        ctypes.c_size_t,
    ]
    lib.axon_start_nrt_profile.restype = ctypes.c_int64
    lib.axon_stop_nrt_profile.argtypes = [ctypes.c_char_p]
    lib.axon_stop_nrt_profile.restype = ctypes.c_int64

    @contextlib.contextmanager
    def _hook(output_dir, device_ids):
        import jax

        jax.devices()
        if device_ids:
            ids = (ctypes.c_int64 * len(device_ids))(*device_ids)
            rc = lib.axon_start_nrt_profile(ids, len(device_ids))
        else:
            rc = lib.axon_start_nrt_profile(None, 0)
        if rc != 0:
            raise RuntimeError(f"axon_start_nrt_profile rc={rc}")
        try:
            yield
        finally:
            n = lib.axon_stop_nrt_profile(str(output_dir).encode())
            print(f"profile: {n} file(s) written to {output_dir}", file=sys.stderr)

    import antenv

    mod = types.ModuleType("antenv.axon_hooks")
    mod.get_axon_ntff_profile_hook = lambda: _hook
    sys.modules["antenv.axon_hooks"] = mod
    antenv.axon_hooks = mod


def kernel(x_out, ref_scores, W, b):
    global LAST_RESULTS
    x_out = np.ascontiguousarray(x_out, dtype=np.float32)
    ref_scores = np.ascontiguousarray(ref_scores, dtype=np.float32)
    W = np.ascontiguousarray(W, dtype=np.float32)
    b = np.ascontiguousarray(b, dtype=np.float32)

    ident = np.eye(128, dtype=np.float32)
    in_maps = []
    for c in range(N_CORES):
        bs = slice(c * B_LOC, (c + 1) * B_LOC)
        in_maps.append({
            "x": x_out[bs].reshape(ROWS, D),
            "scores": ref_scores[bs],
            "w": W,
            "bias": b.reshape(F, 1),
            "ident": ident,
        })

    trace = bool(int(os.environ.get("BASS_KERNEL_TRACE", "0")))
    if trace:
        _install_profile_shim()
    nc = _get_nc()
    res = run_bass_kernel_spmd(
        nc,
        in_maps,
        list(range(N_CORES)),
        trace=trace,
        tmpdir=os.environ.get("BASS_KERNEL_TRACE_DIR") or None,
    )
    LAST_RESULTS = res

    def _unpermute_fcn(fcn_t):
        # device column order is (TP, h, g, p) <-> row TP*1024 + p*8 + h*4 + g
        a = fcn_t.reshape(F, N_GROUPS // 2, 2, GROUP, 128)
        return np.ascontiguousarray(
            a.transpose(1, 4, 2, 3, 0).reshape(ROWS, F)
        )

    fcn = np.concatenate(
        [_unpermute_fcn(res.results[c]["fcn_t"]) for c in range(N_CORES)], axis=0
    ).reshape(B, N, F)
    bbox = np.concatenate(
        [res.results[c]["bbox_t"].T for c in range(N_CORES)], axis=0
    )
    idx = np.concatenate(
        [res.results[c]["idx"][:, 0] for c in range(N_CORES)], axis=0
    ).astype(np.int32)
    slice_inds = np.stack([np.arange(B, dtype=np.int32), idx], axis=1)
    return bbox, fcn, slice_inds
